# revision 25
# baseline (speedup 1.0000x reference)
"""Haar DWT (2x2 stride-2 block decomposition) on 8 Trainium2 NeuronCores.

Input x: (32, 3, 512, 512) f32. Outputs (ll, lh, hl, hh): each (32, 3, 256, 256).

Sharding: pure data parallel over the batch dim — 4 images per core, viewed as
12 channel images of 512x512 per core, one channel per iteration.

The problem is HBM-bandwidth bound, so all device I/O is bf16: the host
downcasts and pre-packs the input (partition-major, even|odd column split —
every DMA is fully contiguous per partition), the device computes the
butterflies exactly in f32 PSUM off bf16 operands, stores bf16, and the host
upcasts. End-to-end error is bf16 rounding of input+output only (~5e-3
max-rel, gate is 2e-2). 12.6 MB/core of HBM traffic vs the ~358 GB/s per-core
limit -> ~35 us roofline plus ~7 us fixed NEFF preamble.

The vertical (row-pair) butterfly runs on the TensorEngine: a constant 128x128
weight matrix W maps 128 image rows to 64 halved row-sums (partitions 0..63)
and 64 halved row-diffs (partitions 64..127) in one 512-column matmul per
128-row tile (4 per channel; the host-packed columns make the moving operand
fully contiguous). Products are +-0.5 * bf16 so the f32 PSUM result is exact.

The horizontal stride-2 column combine stays PER TILE (fine granularity keeps
8 single-bank PSUM tiles in flight, which measured faster than per-channel
ops gated on a 2-deep 4-bank PSUM pipeline): ACT stages the even columns into
SBUF (DVE tensor_tensor can read only one PSUM operand, at 1x), DVE computes
even+odd -> ll|lh and odd-even -> hl|hh as bf16.

DMA queues: all loads on the single SWDGE (gpsimd) queue, stores on the qSP
HWDGE ring issued from the idle sync engine — one queue per direction measured
fastest (more queues just split the SDMA round-robin share and delay the head
of the pipeline), and half-channel stores from sync start the store stream
right after the first two tiles finish instead of queueing behind ACT copies.
"""

import sys

import numpy as np
import ml_dtypes

if "/opt/trn_rl_repo" not in sys.path:
    sys.path.insert(0, "/opt/trn_rl_repo")

from concourse import bacc, bass, mybir
from concourse import tile
from concourse.bass_utils import run_bass_kernel_spmd

N_CORES = 8
B, C, H, W = 32, 3, 512, 512
BPC = B // N_CORES  # images per core
NCH = BPC * C  # channel images per core (12)
P = 128  # SBUF partitions
NT = H // P  # 128-row tiles per channel (4)
HW_OUT = H // 2  # 256

BF16 = ml_dtypes.bfloat16

_CACHE = {}


def _butterfly_weights():
    """W[k, m]: m<64 -> 0.5*(row 2m + row 2m+1); m>=64 -> 0.5*(row 2m'+1 - row 2m')."""
    w = np.zeros((P, P), dtype=np.float32)
    for m in range(64):
        w[2 * m, m] = 0.5
        w[2 * m + 1, m] = 0.5
        w[2 * m, 64 + m] = -0.5
        w[2 * m + 1, 64 + m] = 0.5
    return w.astype(BF16)


def _build():
    nc = bacc.Bacc("TRN2", target_bir_lowering=False, debug=False)
    f32 = mybir.dt.float32
    bf16 = mybir.dt.bfloat16
    # x host-packed as [ch, p, t, even|odd, j]: per-partition bytes are fully
    # contiguous, and each 512-col tile slice is already [even 256 | odd 256]
    x = nc.dram_tensor("x", [NCH, P, NT, W], bf16, kind="ExternalInput")
    w = nc.dram_tensor("w", [P, P], bf16, kind="ExternalInput")
    # out[ch, p, g, t, j]: p<64,g=0: ll row 64t+p | p>=64,g=0: lh row 64t+p-64
    #                      p<64,g=1: hl          | p>=64,g=1: hh
    out = nc.dram_tensor("out", [NCH, P, 2, NT, HW_OUT], bf16, kind="ExternalOutput")
    xa = x.ap()
    oa = out.ap()
    with tile.TileContext(nc) as tc:
        with (
            tc.tile_pool(name="xi", bufs=6) as xpool,
            tc.tile_pool(name="oo", bufs=6) as opool,
            tc.tile_pool(name="cp", bufs=6) as cpool,
            tc.tile_pool(name="w", bufs=1) as wpool,
            tc.tile_pool(name="ps", bufs=2, space=bass.MemorySpace.PSUM) as psum,
        ):
            wt = wpool.tile([P, P], bf16)
            # w rides the otherwise-empty qAct ring so it does not delay the
            # first x quarters on the SWDGE queue or the stores on qSP
            nc.scalar.dma_start(out=wt[:], in_=w.ap())
            for i in range(NCH):
                xin = xpool.tile([P, NT, W], bf16)
                if i == 0:
                    # split the first load so matmuls start earlier
                    for t in range(NT):
                        nc.gpsimd.dma_start(out=xin[:, t, :], in_=xa[i, :, t])
                else:
                    # single SWDGE queue: splitting loads across two DGE rings
                    # measured slower (SDMA round-robin starves the store ring
                    # and delays the first channel's quarters)
                    nc.gpsimd.dma_start(out=xin[:], in_=xa[i])
                outt = opool.tile([P, 2, NT, HW_OUT], bf16)
                pt = psum.tile([P, NT, W], f32)
                for t in range(NT):
                    # columns already packed [even 256 | odd 256]: PSUM result
                    # is [P, even|odd] contiguously; combines are unit-stride
                    nc.tensor.matmul(
                        pt[:, t, :], wt[:], xin[:, t, :], start=True, stop=True
                    )
                pv = pt[:].rearrange("p t (two j) -> p t two j", two=2)
                # ACT stages both halves f32 PSUM -> bf16 SBUF (releasing the
                # PSUM tile), so DVE's combines run bf16 SBUF->SBUF dense at
                # the 2x packed rate instead of PSUM-capped 1x fp32
                cpe = cpool.tile([P, NT, HW_OUT], bf16)
                cpo = cpool.tile([P, NT, HW_OUT], bf16)
                nc.scalar.copy(cpe[:], pv[:, :, 0, :])
                nc.scalar.copy(cpo[:], pv[:, :, 1, :])
                nc.vector.tensor_add(outt[:, 0], cpo[:], cpe[:])
                # per-band stores from the otherwise-idle SP (sync) engine:
                # ll|lh ships while DVE still computes hl|hh
                nc.sync.dma_start(out=oa[i, :, 0], in_=outt[:, 0])
                nc.vector.tensor_sub(outt[:, 1], cpo[:], cpe[:])
                nc.sync.dma_start(out=oa[i, :, 1], in_=outt[:, 1])
    nc.compile()
    return nc


def _get_nc():
    if "nc" not in _CACHE:
        _CACHE["nc"] = _build()
    return _CACHE["nc"]


def _pack_input(x):
    """(32,3,512,512) f32 -> (cores, NCH, P, NT*W) bf16, partition-major.

    Row r = 128t + p, col c = 2j + e; packed order [core][ch][p][t][e][j] so
    each channel load is one fully contiguous 512 KB DMA and each 512-col tile
    slice is [even 256 | odd 256].
    """
    xb = np.asarray(x, dtype=np.float32).astype(BF16)
    xb = xb.reshape(N_CORES, NCH, NT, P, HW_OUT, 2)
    xb = xb.transpose(0, 1, 3, 2, 5, 4)  # [core][ch][p][t][e][j]
    return np.ascontiguousarray(xb).reshape(N_CORES, NCH, P, NT * W)


def run(x, **spmd_kwargs):
    """Run the DWT on 8 cores; returns (results_tuple, BassKernelResults)."""
    nc = _get_nc()
    xs = _pack_input(x)
    wmat = _butterfly_weights()
    in_maps = [{"x": xs[i].reshape(NCH, P, NT, W), "w": wmat} for i in range(N_CORES)]
    res = None
    for attempt in range(3):
        try:
            res = run_bass_kernel_spmd(
                nc, in_maps, core_ids=list(range(N_CORES)), **spmd_kwargs
            )
            break
        except Exception:
            # transient device wedge (NRT_EXEC_UNIT_UNRECOVERABLE) recovers
            # on retry; re-raise only if it persists
            if attempt == 2:
                raise
            import time

            time.sleep(2)
    # per-core out: (NCH, P, 2, NT, HW_OUT) bf16
    full = np.stack([np.asarray(res.results[i]["out"]) for i in range(N_CORES)])
    # -> (cores, NCH, NT, P, 2, j): out image row r = 64*t + (p mod 64)
    full = full.transpose(0, 1, 4, 2, 3, 5)

    def expand(sl):  # (cores, NCH, NT, 64, j) -> (B, C, 256, 256)
        return (
            np.ascontiguousarray(sl)
            .astype(np.float32)
            .reshape(B, C, HW_OUT, HW_OUT)
        )

    ll = expand(full[:, :, :, 0:64, 0, :])
    lh = expand(full[:, :, :, 64:128, 0, :])
    hl = expand(full[:, :, :, 0:64, 1, :])
    hh = expand(full[:, :, :, 64:128, 1, :])
    return (ll, lh, hl, hh), res


def kernel(x):
    out, _ = run(x)
    return out


# revision 26
# speedup vs baseline: 1.0715x; 1.0715x over previous
"""Haar DWT (2x2 stride-2 block decomposition) on 8 Trainium2 NeuronCores.

Input x: (32, 3, 512, 512) f32. Outputs (ll, lh, hl, hh): each (32, 3, 256, 256).

Sharding: pure data parallel over the batch dim — 4 images per core, viewed as
12 channel images of 512x512 per core, one channel per iteration.

The problem is HBM-bandwidth bound, so all device I/O is bf16: the host
downcasts and pre-packs the input (partition-major, even|odd column split —
every DMA is fully contiguous per partition), the device computes the
butterflies exactly in f32 PSUM off bf16 operands, stores bf16, and the host
upcasts. End-to-end error is bf16 rounding of input+output only (~5e-3
max-rel, gate is 2e-2). 12.6 MB/core of HBM traffic vs the ~420 GB/s combined
DMA wall measured on-core -> ~30 us data time plus ~7 us fixed NEFF preamble.

The vertical (row-pair) butterfly runs on the TensorEngine: a constant 128x128
weight matrix W maps 128 image rows to 64 halved row-sums (partitions 0..63)
and 64 halved row-diffs (partitions 64..127) in one 512-column matmul per
128-row tile (4 per channel; the host-packed columns make the moving operand
fully contiguous). Products are +-0.5 * bf16 so the f32 PSUM result is exact.

The horizontal stride-2 column combine stays PER TILE: fine granularity keeps
8 single-bank PSUM tiles in flight, which measured faster than any per-channel
variant (coarse PSUM tiles stall the matmul stream on PSUM recycling; staging
both halves to SBUF for 2x bf16 DVE ops just moves the wall to ACT). ACT
stages the even columns into SBUF (DVE tensor_tensor reads at most one PSUM
operand, at 1x), DVE computes even+odd -> ll|lh and odd-even -> hl|hh as bf16.

DMA queues: all loads on the single SWDGE (gpsimd) queue, stores on the qAct
HWDGE ring (scalar) — one queue per direction measured fastest; splitting
either direction across two DGE rings starves the SDMA round-robin share of
the other direction and delays the first channel (measured +3.5 us).
"""

import sys

import numpy as np
import ml_dtypes

if "/opt/trn_rl_repo" not in sys.path:
    sys.path.insert(0, "/opt/trn_rl_repo")

from concourse import bacc, bass, mybir
from concourse import tile
from concourse.bass_utils import run_bass_kernel_spmd

N_CORES = 8
B, C, H, W = 32, 3, 512, 512
BPC = B // N_CORES  # images per core
NCH = BPC * C  # channel images per core (12)
P = 128  # SBUF partitions
NT = H // P  # 128-row tiles per channel (4)
HW_OUT = H // 2  # 256

BF16 = ml_dtypes.bfloat16

_CACHE = {}


def _butterfly_weights():
    """W[k, m]: m<64 -> 0.5*(row 2m + row 2m+1); m>=64 -> 0.5*(row 2m'+1 - row 2m')."""
    w = np.zeros((P, P), dtype=np.float32)
    for m in range(64):
        w[2 * m, m] = 0.5
        w[2 * m + 1, m] = 0.5
        w[2 * m, 64 + m] = -0.5
        w[2 * m + 1, 64 + m] = 0.5
    return w.astype(BF16)


def _build():
    nc = bacc.Bacc("TRN2", target_bir_lowering=False, debug=False)
    f32 = mybir.dt.float32
    bf16 = mybir.dt.bfloat16
    # x host-packed as [ch, p, t, even|odd, j]: per-partition bytes are fully
    # contiguous, and each 512-col tile slice is already [even 256 | odd 256]
    x = nc.dram_tensor("x", [NCH, P, NT, W], bf16, kind="ExternalInput")
    w = nc.dram_tensor("w", [P, P], bf16, kind="ExternalInput")
    # out[ch, p, t, g, j]: p<64,g=0: ll row 64t+p | p>=64,g=0: lh row 64t+p-64
    #                      p<64,g=1: hl          | p>=64,g=1: hh
    out = nc.dram_tensor("out", [NCH, P, NT, 2, HW_OUT], bf16, kind="ExternalOutput")
    xa = x.ap()
    oa = out.ap()
    with tile.TileContext(nc) as tc:
        with (
            tc.tile_pool(name="xi", bufs=6) as xpool,
            tc.tile_pool(name="oo", bufs=6) as opool,
            tc.tile_pool(name="cp", bufs=8) as cpool,
            tc.tile_pool(name="w", bufs=1) as wpool,
            tc.tile_pool(name="ps", bufs=8, space=bass.MemorySpace.PSUM) as psum,
        ):
            wt = wpool.tile([P, P], bf16)
            nc.sync.dma_start(out=wt[:], in_=w.ap())
            for i in range(NCH):
                xin = xpool.tile([P, NT, W], bf16)
                if i == 0:
                    # split the first load so matmuls start earlier
                    for t in range(NT):
                        nc.gpsimd.dma_start(out=xin[:, t, :], in_=xa[i, :, t])
                else:
                    nc.gpsimd.dma_start(out=xin[:], in_=xa[i])
                outt = opool.tile([P, NT, 2, HW_OUT], bf16)
                for t in range(NT):
                    pt = psum.tile([P, W], f32)
                    # columns already packed [even 256 | odd 256]: PSUM result
                    # is [P, even|odd] contiguously; combines are unit-stride
                    nc.tensor.matmul(pt[:], wt[:], xin[:, t, :], start=True, stop=True)
                    pv = pt[:].rearrange("p (two j) -> p two j", two=2)
                    # DVE can read at most one PSUM operand per instruction:
                    # ACT (otherwise idle) stages the even columns into SBUF.
                    cp = cpool.tile([P, HW_OUT], f32)
                    nc.scalar.copy(cp[:], pv[:, 0, :])
                    nc.vector.tensor_add(outt[:, t, 0], pv[:, 1, :], cp[:])
                    nc.vector.tensor_sub(outt[:, t, 1], pv[:, 1, :], cp[:])
                if i == NCH - 1:
                    # split the last store so the tail drains in halves
                    nc.scalar.dma_start(out=oa[i, :, 0:2], in_=outt[:, 0:2])
                    nc.scalar.dma_start(out=oa[i, :, 2:4], in_=outt[:, 2:4])
                else:
                    nc.scalar.dma_start(out=oa[i], in_=outt[:])
    nc.compile()
    return nc


def _get_nc():
    if "nc" not in _CACHE:
        _CACHE["nc"] = _build()
    return _CACHE["nc"]


def _pack_input(x):
    """(32,3,512,512) f32 -> (cores, NCH, P, NT*W) bf16, partition-major.

    Row r = 128t + p, col c = 2j + e; packed order [core][ch][p][t][e][j] so
    each channel load is one fully contiguous 512 KB DMA and each 512-col tile
    slice is [even 256 | odd 256].
    """
    xb = np.asarray(x, dtype=np.float32).astype(BF16)
    xb = xb.reshape(N_CORES, NCH, NT, P, HW_OUT, 2)
    xb = xb.transpose(0, 1, 3, 2, 5, 4)  # [core][ch][p][t][e][j]
    return np.ascontiguousarray(xb).reshape(N_CORES, NCH, P, NT * W)


def run(x, **spmd_kwargs):
    """Run the DWT on 8 cores; returns (results_tuple, BassKernelResults)."""
    nc = _get_nc()
    xs = _pack_input(x)
    wmat = _butterfly_weights()
    in_maps = [{"x": xs[i].reshape(NCH, P, NT, W), "w": wmat} for i in range(N_CORES)]
    res = None
    for attempt in range(3):
        try:
            res = run_bass_kernel_spmd(
                nc, in_maps, core_ids=list(range(N_CORES)), **spmd_kwargs
            )
            break
        except Exception:
            # transient device wedge (NRT_EXEC_UNIT_UNRECOVERABLE) recovers
            # on retry; re-raise only if it persists
            if attempt == 2:
                raise
            import time

            time.sleep(2)
    # per-core out: (NCH, P, NT, 2, HW_OUT) bf16
    full = np.stack([np.asarray(res.results[i]["out"]) for i in range(N_CORES)])
    # -> (cores, NCH, NT, P, 2, j): out image row r = 64*t + (p mod 64)
    full = full.transpose(0, 1, 3, 2, 4, 5)

    def expand(sl):  # (cores, NCH, NT, 64, j) -> (B, C, 256, 256)
        return (
            np.ascontiguousarray(sl)
            .astype(np.float32)
            .reshape(B, C, HW_OUT, HW_OUT)
        )

    ll = expand(full[:, :, :, 0:64, 0, :])
    lh = expand(full[:, :, :, 64:128, 0, :])
    hl = expand(full[:, :, :, 0:64, 1, :])
    hh = expand(full[:, :, :, 64:128, 1, :])
    return (ll, lh, hl, hh), res


def kernel(x):
    out, _ = run(x)
    return out


# revision 28
# speedup vs baseline: 1.1785x; 1.0998x over previous
"""Haar DWT (2x2 stride-2 block decomposition) on 8 Trainium2 NeuronCores.

Input x: (32, 3, 512, 512) f32. Outputs (ll, lh, hl, hh): each (32, 3, 256, 256).

Sharding: pure data parallel over the batch dim — 4 images per core, viewed as
12 channel images of 512x512 per core, one channel per iteration.

The problem is HBM-bandwidth bound, so all device I/O is bf16: the host
downcasts and pre-packs the input (partition-major, even|odd column split —
every DMA is fully contiguous per partition), the device computes the
butterflies exactly in f32 PSUM off bf16 operands, stores bf16, and the host
upcasts. End-to-end error is bf16 rounding of input+output only (~5e-3
max-rel, gate is 2e-2). 12.6 MB/core of HBM traffic vs the ~420 GB/s combined
DMA wall measured on-core -> ~30 us data time plus ~7 us fixed NEFF preamble.

The vertical (row-pair) butterfly runs on the TensorEngine: a constant 128x128
weight matrix W maps 128 image rows to 64 halved row-sums (partitions 0..63)
and 64 halved row-diffs (partitions 64..127) in one 512-column matmul per
128-row tile (4 per channel; the host-packed columns make the moving operand
fully contiguous). Products are +-0.5 * bf16 so the f32 PSUM result is exact.

The horizontal stride-2 column combine stays PER TILE: fine granularity keeps
8 single-bank PSUM tiles in flight, which measured faster than any per-channel
variant (coarse PSUM tiles stall the matmul stream on PSUM recycling; staging
both halves to SBUF for 2x bf16 DVE ops just moves the wall to ACT). ACT
stages the even columns into SBUF (DVE tensor_tensor reads at most one PSUM
operand, at 1x), DVE computes even+odd -> ll|lh and odd-even -> hl|hh as bf16.

DMA queues: all loads on the single SWDGE (gpsimd) queue, stores on the qAct
HWDGE ring (scalar) — one queue per direction measured fastest; splitting
either direction across two DGE rings starves the SDMA round-robin share of
the other direction and delays the first channel (measured +3.5 us).
"""

import sys

import numpy as np
import ml_dtypes

if "/opt/trn_rl_repo" not in sys.path:
    sys.path.insert(0, "/opt/trn_rl_repo")

from concourse import bacc, bass, mybir
from concourse import tile
from concourse.bass_utils import run_bass_kernel_spmd

N_CORES = 8
B, C, H, W = 32, 3, 512, 512
BPC = B // N_CORES  # images per core
NCH = BPC * C  # channel images per core (12)
P = 128  # SBUF partitions
NT = H // P  # 128-row tiles per channel (4)
HW_OUT = H // 2  # 256

BF16 = ml_dtypes.bfloat16

_CACHE = {}


def _butterfly_weights():
    """W[k, m]: m<64 -> 0.5*(row 2m + row 2m+1); m>=64 -> 0.5*(row 2m'+1 - row 2m')."""
    w = np.zeros((P, P), dtype=np.float32)
    for m in range(64):
        w[2 * m, m] = 0.5
        w[2 * m + 1, m] = 0.5
        w[2 * m, 64 + m] = -0.5
        w[2 * m + 1, 64 + m] = 0.5
    return w.astype(BF16)


def _build():
    nc = bacc.Bacc("TRN2", target_bir_lowering=False, debug=False)
    f32 = mybir.dt.float32
    bf16 = mybir.dt.bfloat16
    # x host-packed as [ch, p, t, even|odd, j]: per-partition bytes are fully
    # contiguous, and each 512-col tile slice is already [even 256 | odd 256]
    x = nc.dram_tensor("x", [NCH, P, NT, W], bf16, kind="ExternalInput")
    w = nc.dram_tensor("w", [P, P], bf16, kind="ExternalInput")
    # out[ch, p, t, g, j]: p<64,g=0: ll row 64t+p | p>=64,g=0: lh row 64t+p-64
    #                      p<64,g=1: hl          | p>=64,g=1: hh
    out = nc.dram_tensor("out", [NCH, P, NT, 2, HW_OUT], bf16, kind="ExternalOutput")
    xa = x.ap()
    oa = out.ap()
    with tile.TileContext(nc) as tc:
        with (
            tc.tile_pool(name="xi", bufs=6) as xpool,
            tc.tile_pool(name="oo", bufs=6) as opool,
            tc.tile_pool(name="cp", bufs=12) as cpool,
            tc.tile_pool(name="w", bufs=1) as wpool,
            tc.tile_pool(name="ps", bufs=8, space=bass.MemorySpace.PSUM) as psum,
        ):
            wt = wpool.tile([P, P], bf16)
            nc.sync.dma_start(out=wt[:], in_=w.ap())
            for i in range(NCH):
                xin = xpool.tile([P, NT, W], bf16)
                if i == 0:
                    # split the first load so matmuls start earlier
                    for t in range(NT):
                        nc.gpsimd.dma_start(out=xin[:, t, :], in_=xa[i, :, t])
                else:
                    nc.gpsimd.dma_start(out=xin[:], in_=xa[i])
                outt = opool.tile([P, NT, 2, HW_OUT], bf16)
                for t in range(NT):
                    pt = psum.tile([P, W], f32)
                    # columns already packed [even 256 | odd 256]: PSUM result
                    # is [P, even|odd] contiguously; combines are unit-stride
                    nc.tensor.matmul(pt[:], wt[:], xin[:, t, :], start=True, stop=True)
                    pv = pt[:].rearrange("p (two j) -> p two j", two=2)
                    # DVE is the pipeline wall (tensor_tensor with a PSUM
                    # operand is capped at 1x fp32). Rebalance: on half the
                    # tiles ACT stages BOTH halves as bf16 so DVE's combines
                    # run bf16 SBUF->SBUF dense at the 2x packed rate
                    # (264 ns vs 415 ns per op); the other half keep the
                    # single f32 staging copy so ACT stays under DVE.
                    if t % 2 == 1:
                        cpe = cpool.tile([P, HW_OUT], bf16)
                        cpo = cpool.tile([P, HW_OUT], bf16)
                        nc.scalar.copy(cpe[:], pv[:, 0, :])
                        nc.scalar.copy(cpo[:], pv[:, 1, :])
                        nc.vector.tensor_add(outt[:, t, 0], cpo[:], cpe[:])
                        nc.vector.tensor_sub(outt[:, t, 1], cpo[:], cpe[:])
                    else:
                        cp = cpool.tile([P, HW_OUT], f32)
                        nc.scalar.copy(cp[:], pv[:, 0, :])
                        nc.vector.tensor_add(outt[:, t, 0], pv[:, 1, :], cp[:])
                        nc.vector.tensor_sub(outt[:, t, 1], pv[:, 1, :], cp[:])
                # stores issue from the otherwise-idle SP (sync) engine so
                # ACT's budget is pure staging work
                if i == NCH - 1:
                    # split the last store so the tail drains in halves
                    nc.sync.dma_start(out=oa[i, :, 0:2], in_=outt[:, 0:2])
                    nc.sync.dma_start(out=oa[i, :, 2:4], in_=outt[:, 2:4])
                else:
                    nc.sync.dma_start(out=oa[i], in_=outt[:])
    nc.compile()
    return nc


def _get_nc():
    if "nc" not in _CACHE:
        _CACHE["nc"] = _build()
    return _CACHE["nc"]


def _pack_input(x):
    """(32,3,512,512) f32 -> (cores, NCH, P, NT*W) bf16, partition-major.

    Row r = 128t + p, col c = 2j + e; packed order [core][ch][p][t][e][j] so
    each channel load is one fully contiguous 512 KB DMA and each 512-col tile
    slice is [even 256 | odd 256].
    """
    xb = np.asarray(x, dtype=np.float32).astype(BF16)
    xb = xb.reshape(N_CORES, NCH, NT, P, HW_OUT, 2)
    xb = xb.transpose(0, 1, 3, 2, 5, 4)  # [core][ch][p][t][e][j]
    return np.ascontiguousarray(xb).reshape(N_CORES, NCH, P, NT * W)


def run(x, **spmd_kwargs):
    """Run the DWT on 8 cores; returns (results_tuple, BassKernelResults)."""
    nc = _get_nc()
    xs = _pack_input(x)
    wmat = _butterfly_weights()
    in_maps = [{"x": xs[i].reshape(NCH, P, NT, W), "w": wmat} for i in range(N_CORES)]
    res = None
    for attempt in range(3):
        try:
            res = run_bass_kernel_spmd(
                nc, in_maps, core_ids=list(range(N_CORES)), **spmd_kwargs
            )
            break
        except Exception:
            # transient device wedge (NRT_EXEC_UNIT_UNRECOVERABLE) recovers
            # on retry; re-raise only if it persists
            if attempt == 2:
                raise
            import time

            time.sleep(2)
    # per-core out: (NCH, P, NT, 2, HW_OUT) bf16
    full = np.stack([np.asarray(res.results[i]["out"]) for i in range(N_CORES)])
    # -> (cores, NCH, NT, P, 2, j): out image row r = 64*t + (p mod 64)
    full = full.transpose(0, 1, 3, 2, 4, 5)

    def expand(sl):  # (cores, NCH, NT, 64, j) -> (B, C, 256, 256)
        return (
            np.ascontiguousarray(sl)
            .astype(np.float32)
            .reshape(B, C, HW_OUT, HW_OUT)
        )

    ll = expand(full[:, :, :, 0:64, 0, :])
    lh = expand(full[:, :, :, 64:128, 0, :])
    hl = expand(full[:, :, :, 0:64, 1, :])
    hh = expand(full[:, :, :, 64:128, 1, :])
    return (ll, lh, hl, hh), res


def kernel(x):
    out, _ = run(x)
    return out
